# revision 1
# baseline (speedup 1.0000x reference)
"""Trainium2 Bass kernel for GQA attention forward (B=2, S=2048, D=2048,
16 q-heads / 4 kv-heads, head_dim=128, RoPE, causal).

Sharding: 8 cores = 2 (batch) x 4 (kv-head groups). Each core computes its
batch's attention for one kv-head group (4 q-heads + 1 kv head) and a
row-parallel partial of the output projection; the host sums the 4 partials
per batch.

Matmul operands are bf16 (1 cycle/row on PE) with fp32 PSUM accumulation;
the softmax denominator path runs in fp32/f32r to avoid bf16 rounding of
the normalization.
"""

import sys

if "/opt/trn_rl_repo" not in sys.path:
    sys.path.insert(0, "/opt/trn_rl_repo")

import numpy as np
import ml_dtypes

import concourse.bass as bass
import concourse.tile as tile
from concourse import mybir

F32 = mybir.dt.float32
F32R = mybir.dt.float32r
BF16 = mybir.dt.bfloat16

# Full-problem constants (per reference).
B, S, DIM = 2, 2048, 2048
N_HEADS, N_KV_HEADS, HEAD_DIM = 16, 4, 128
N_GROUPS = N_KV_HEADS          # tensor-parallel groups
HQ = N_HEADS // N_KV_HEADS     # q heads per group
NEG = -1e30


def build_attention_core(nc, S=S, D=DIM, HQ=HQ, HD=HEAD_DIM, CHUNK=512):
    """Emit the per-core attention program into `nc` (Tile framework).

    Inputs (ExternalInput dram tensors):
      x      [S, D]  bf16   activations for this core's batch
      wqT    [D, HQ*HD] bf16  q projection, transposed, RoPE-permuted rows
      wkvT   [D, 2*HD] bf16   [wk^T | wv^T] (wk RoPE-permuted)
      woT    [HQ*HD, D] bf16  output projection slice, transposed
      t1,t2  [S, HD] f32      RoPE tables (permuted-half layout)
      masks  [CHUNK//128, 128, CHUNK] f32 additive causal masks
      ident  [128, 128] bf16  identity for PE transposes
      ones_col [128,1] bf16 / ones_row [1,128] f32r
    Output:
      out_partial [S, D] f32
    """
    n_st = S // 128        # s tiles
    n_dt = D // 128        # d tiles
    n_ch = S // CHUNK      # q chunks
    kpc = CHUNK // 128     # k-tiles per chunk
    n_dc = D // CHUNK      # d chunks (phase C)
    IQ = HQ * HD

    x_d = nc.dram_tensor("xT", [128, D // 128, S], BF16, kind="ExternalInput")
    wqT_d = nc.dram_tensor("wqT", [128, D // 128, IQ], BF16, kind="ExternalInput")
    wkvT_d = nc.dram_tensor("wkvT", [128, D // 128, 2 * HD], BF16, kind="ExternalInput")
    woT_d = nc.dram_tensor("woT", [128, IQ // 128, D], BF16, kind="ExternalInput")
    t1_d = nc.dram_tensor("t1", [128, S // 128, HD], F32, kind="ExternalInput")
    t2_d = nc.dram_tensor("t2", [128, S // 128, HD], F32, kind="ExternalInput")
    masks_d = nc.dram_tensor("masks", [128, kpc, CHUNK], F32, kind="ExternalInput")
    ident_d = nc.dram_tensor("ident", [128, 128], BF16, kind="ExternalInput")
    onesc_d = nc.dram_tensor("ones_col", [128, 1], BF16, kind="ExternalInput")
    onesr_d = nc.dram_tensor("ones_row", [1, 128], F32R, kind="ExternalInput")
    out_d = nc.dram_tensor("out_partial", [S, D], F32, kind="ExternalOutput")

    scale = float(HD) ** -0.5

    with tile.TileContext(nc) as tc:
        with (
            # tensors persistent across phases
            tc.tile_pool(name="persist", bufs=1) as persist,
            tc.tile_pool(name="constB", bufs=1) as constB,
        ):
            qT_sb = persist.tile([128, HQ, S], BF16)    # [e, h, s]
            kT_sb = persist.tile([128, S], BF16)        # [e, s]
            v_sb = persist.tile([128, n_st, HD], BF16)  # [s_in_tile, s_tile, e]
            oT_sb = persist.tile([128, HQ, S], BF16)    # [e, h, s]

            # ---------------- Phase A: projections + RoPE -------------------
            with (
                tc.tile_pool(name="weightsA", bufs=1) as weightsA,
                tc.tile_pool(name="xt", bufs=1) as xt_pool,
                tc.tile_pool(name="rope", bufs=4) as rope_pool,
                tc.tile_pool(name="ps_t", bufs=3, space="PSUM") as pst_pool,
                tc.tile_pool(name="ps_q", bufs=2, space="PSUM") as psq_pool,
                tc.tile_pool(name="ps_kv", bufs=2, space="PSUM") as pskv_pool,
            ):
                gq = n_dt // 4
                wq_g = []
                wkv_g = []
                for g in range(4):
                    wqg = weightsA.tile([128, gq, IQ], BF16, tag=f"wq{g}",
                                        name=f"wq{g}")
                    nc.scalar.dma_start(
                        out=wqg, in_=wqT_d[:, g * gq:(g + 1) * gq, :]
                    )
                    wq_g.append(wqg)
                    wkvg = weightsA.tile([128, gq, 2 * HD], BF16, tag=f"wkv{g}",
                                         name=f"wkv{g}")
                    nc.scalar.dma_start(
                        out=wkvg, in_=wkvT_d[:, g * gq:(g + 1) * gq, :]
                    )
                    wkv_g.append(wkvg)
                ident = weightsA.tile([128, 128], BF16)
                nc.scalar.dma_start(out=ident, in_=ident_d[:])
                t1_sb = weightsA.tile([128, n_st, HD], F32)
                nc.scalar.dma_start(
                    out=t1_sb, in_=t1_d[:]
                )
                t2_sb = weightsA.tile([128, n_st, HD], F32)
                nc.scalar.dma_start(
                    out=t2_sb, in_=t2_d[:]
                )

                # pre-transposed activations: 2 d-tiles per DMA, sync queue
                xpair = []
                for g2 in range(n_dt // 2):
                    xt_t = xt_pool.tile([128, 2, S], BF16, tag=f"xt{g2}")
                    nc.sync.dma_start(out=xt_t, in_=x_d[:, g2 * 2:g2 * 2 + 2, :])
                    xpair.append(xt_t)
                xT = [xpair[dt_ // 2][:, dt_ % 2, :] for dt_ in range(n_dt)]

                # phase-B constants loaded early so the first diagonal
                # mask-add never waits
                masks_sb = constB.tile([128, kpc, CHUNK], F32)
                nc.sync.dma_start(out=masks_sb, in_=masks_d[:])
                ones_col = constB.tile([128, 1], BF16)
                nc.sync.dma_start(out=ones_col, in_=onesc_d[:])
                ones_row = constB.tile([1, 128], F32R)
                nc.sync.dma_start(out=ones_row, in_=onesr_d[:])

                rope_pending = None

                def emit_transposes(rp):
                    q_rot_, k_rot_, sl_ = rp
                    for h in range(HQ):
                        ps_tq = pst_pool.tile([128, 128], BF16, tag="ps_t")
                        nc.tensor.transpose(
                            ps_tq, q_rot_[:, h * HD:(h + 1) * HD], ident
                        )
                        nc.vector.tensor_copy(qT_sb[:, h, sl_], ps_tq)
                    ps_tk = pst_pool.tile([128, 128], BF16, tag="ps_t")
                    nc.tensor.transpose(ps_tk, k_rot_, ident)
                    nc.vector.tensor_copy(kT_sb[:, sl_], ps_tk)

                for st in range(n_st):
                    ps_q = psq_pool.tile([128, IQ], F32)
                    ps_kv = pskv_pool.tile([128, 2 * HD], F32)
                    st_sl = slice(st * 128, (st + 1) * 128)
                    for dt_ in range(n_dt):
                        nc.tensor.matmul(
                            ps_kv, xT[dt_][:, st_sl], wkv_g[dt_ // gq][:, dt_ % gq, :],
                            start=(dt_ == 0), stop=(dt_ == n_dt - 1),
                        )
                        nc.tensor.matmul(
                            ps_q, xT[dt_][:, st_sl], wq_g[dt_ // gq][:, dt_ % gq, :],
                            start=(dt_ == 0), stop=(dt_ == n_dt - 1),
                        )
                    # previous s-tile's PE transposes: emitted here so PE
                    # never waits on the DVE RoPE chain
                    if rope_pending is not None:
                        emit_transposes(rope_pending)
                        rope_pending = None

                    # RoPE on all q heads at once (tables broadcast
                    # across heads via zero-stride AP)
                    t1s = t1_sb[:, st, :]
                    t2s = t2_sb[:, st, :]
                    t1b = bass.AP(tensor=t1s.tensor, offset=t1s.offset,
                                  ap=[t1s.ap[0], [0, HQ], t1s.ap[1]])
                    t2b = bass.AP(tensor=t2s.tensor, offset=t2s.offset,
                                  ap=[t2s.ap[0], [0, HQ], t2s.ap[1]])
                    ps_qv = ps_q.rearrange("p (h e) -> p h e", h=HQ)
                    t1m = rope_pool.tile([128, HQ, HD], F32, tag="t1m")
                    nc.vector.tensor_mul(t1m, ps_qv, t1b)
                    t2m = rope_pool.tile([128, HQ, HD], F32, tag="t2m")
                    nc.vector.tensor_mul(
                        t2m[:, :, 0:64], ps_qv[:, :, 64:128], t2b[:, :, 0:64]
                    )
                    nc.vector.tensor_mul(
                        t2m[:, :, 64:128], ps_qv[:, :, 0:64], t2b[:, :, 64:128]
                    )
                    q_rot = rope_pool.tile([128, HQ * HD], BF16, tag="qrot")
                    nc.vector.tensor_add(
                        q_rot.rearrange("p (h e) -> p h e", h=HQ), t1m, t2m
                    )
                    # RoPE on k
                    t1mk = rope_pool.tile([128, HD], F32, tag="t1mk")
                    nc.vector.tensor_mul(t1mk, ps_kv[:, 0:HD], t1_sb[:, st, :])
                    t2mk = rope_pool.tile([128, HD], F32, tag="t2mk")
                    nc.vector.tensor_mul(
                        t2mk[:, 0:64], ps_kv[:, 64:128], t2_sb[:, st, 0:64]
                    )
                    nc.vector.tensor_mul(
                        t2mk[:, 64:128], ps_kv[:, 0:64], t2_sb[:, st, 64:128]
                    )
                    k_rot = rope_pool.tile([128, HD], BF16, tag="krot")
                    nc.vector.tensor_add(k_rot, t1mk, t2mk)
                    rope_pending = (q_rot, k_rot, st_sl)

                    # v: straight copy (natural [s, e] layout), cast to bf16
                    nc.scalar.copy(v_sb[:, st, :], ps_kv[:, HD:2 * HD])
                emit_transposes(rope_pending)

            # ---------------- Phases B+C shared: woT ------------------------
            with tc.tile_pool(name="weightsC", bufs=1) as weightsC:
                woT_sb = weightsC.tile([128, IQ // 128, D], BF16)
                nc.sync.dma_start(
                    out=woT_sb, in_=woT_d[:]
                )

                # ---------------- Phase B: attention ------------------------
                with (
                    tc.tile_pool(name="expt", bufs=4) as expt_pool,
                    tc.tile_pool(name="maskbuf", bufs=3) as mask_pool,
                    tc.tile_pool(name="sums", bufs=2) as sums_pool,
                    tc.tile_pool(name="recip", bufs=2) as rec_pool,
                    tc.tile_pool(name="ps_s", bufs=3, space="PSUM") as pss_pool,
                    tc.tile_pool(name="ps_o", bufs=2, space="PSUM") as pso_pool,
                    tc.tile_pool(name="ps_sum", bufs=2, space="PSUM") as pssum_pool,
                    tc.tile_pool(name="ps_b", bufs=1, space="PSUM") as psb_pool,
                ):
                    norm_pending = [None]

                    def emit_norm():
                        ps_o_, ps_sum_, h_, c_ = norm_pending[0]
                        norm_pending[0] = None
                        sums_sb = sums_pool.tile([1, CHUNK], F32R, tag="sums")
                        with nc.allow_low_precision(reason="f32r denom"):
                            nc.vector.tensor_copy(sums_sb, ps_sum_)
                        ps_b = psb_pool.tile([128, CHUNK], F32)
                        nc.tensor.matmul(
                            ps_b, ones_row, sums_sb, start=True, stop=True,
                        )
                        recip = rec_pool.tile([128, CHUNK], F32)
                        nc.vector.reciprocal_approx_fast(recip, ps_b)
                        nc.vector.tensor_mul(
                            oT_sb[:, h_, c_ * CHUNK:(c_ + 1) * CHUNK],
                            ps_o_, recip,
                        )

                    for h in range(HQ):
                        for c in range(n_ch):
                            ps_o = pso_pool.tile([128, CHUNK], F32)
                            ps_sum = pssum_pool.tile([1, CHUNK], F32)
                            n_kj = (c + 1) * kpc
                            c_sl = slice(c * CHUNK, (c + 1) * CHUNK)
                            pending = []

                            def flush_one():
                                pe, pj, poff = pending.pop(0)
                                nc.tensor.matmul(
                                    ps_o[:, poff:], v_sb[:, pj, :], pe,
                                    start=(pj == 0), stop=(pj == n_kj - 1),
                                )
                                nc.tensor.matmul(
                                    ps_sum[:, poff:], ones_col, pe,
                                    start=(pj == 0), stop=(pj == n_kj - 1),
                                )

                            for kj in range(n_kj):
                                # columns left of the diagonal block are fully
                                # masked: skip them (q >= kj*128 only)
                                off = max(0, (kj - c * kpc)) * 128
                                w = CHUNK - off
                                ps_s = pss_pool.tile([128, CHUNK], F32, tag="ps_s")
                                nc.tensor.matmul(
                                    ps_s[:, 0:w],
                                    kT_sb[:, kj * 128:(kj + 1) * 128],
                                    qT_sb[:, h, c * CHUNK + off:(c + 1) * CHUNK],
                                    start=True, stop=True,
                                )
                                if kj >= c * kpc:  # diagonal chunk: causal mask
                                    msk = mask_pool.tile([128, CHUNK], F32, tag="msk")
                                    nc.vector.tensor_add(
                                        msk[:, 0:w], ps_s[:, 0:w],
                                        masks_sb[:, kj % kpc, off:],
                                    )
                                    exp_in = msk
                                else:
                                    exp_in = ps_s
                                expT = expt_pool.tile([128, CHUNK], BF16, tag="expT")
                                nc.scalar.activation(
                                    expT[:, 0:w], exp_in[:, 0:w],
                                    mybir.ActivationFunctionType.Exp,
                                    scale=scale,
                                )
                                pending.append((expT[:, 0:w], kj, off))
                                if kj == 1 and norm_pending[0] is not None:
                                    emit_norm()
                                if len(pending) > 2:
                                    flush_one()
                            while pending:
                                flush_one()
                            norm_pending[0] = (ps_o, ps_sum, h, c)

                    emit_norm()

                # ---------------- Phase C: output projection ----------------
                with (
                    tc.tile_pool(name="outsb", bufs=2) as outsb_pool,
                    tc.tile_pool(name="ps_d", bufs=4, space="PSUM") as psd_pool,
                ):
                    for st in range(n_st):
                        out_sb = outsb_pool.tile([128, D], F32)
                        for dc in range(n_dc):
                            ps_d = psd_pool.tile([128, CHUNK], F32)
                            for it in range(HQ):
                                nc.tensor.matmul(
                                    ps_d,
                                    oT_sb[:, it, st * 128:(st + 1) * 128],
                                    woT_sb[:, it, dc * CHUNK:(dc + 1) * CHUNK],
                                    start=(it == 0), stop=(it == HQ - 1),
                                )
                            nc.scalar.copy(
                                out_sb[:, dc * CHUNK:(dc + 1) * CHUNK], ps_d
                            )
                        nc.sync.dma_start(
                            out=out_d[st * 128:(st + 1) * 128, :], in_=out_sb
                        )

    return nc


# ---------------------------------------------------------------------------
# Host-side prep


_ROPE_PERM = np.concatenate([np.arange(0, HEAD_DIM, 2), np.arange(1, HEAD_DIM, 2)])


def _prep_tables(freq_cis, S_=S, HD_=HEAD_DIM):
    """RoPE tables in permuted-half layout: rot = q*t1 + swap(q)*t2."""
    fc = np.asarray(freq_cis, dtype=np.float32)
    A = fc[:, :, 0, 0]
    Bm = fc[:, :, 0, 1]
    C = fc[:, :, 1, 0]
    Dm = fc[:, :, 1, 1]
    t1 = np.concatenate([A, Dm], axis=1).astype(np.float32)  # [S, HD]
    t2 = np.concatenate([Bm, C], axis=1).astype(np.float32)
    return np.ascontiguousarray(t1), np.ascontiguousarray(t2)


def _prep_masks(chunk=512):
    kpc = chunk // 128
    masks = np.zeros((kpc, 128, chunk), dtype=np.float32)
    q = np.arange(chunk)[None, :]
    p = np.arange(128)[:, None]
    for j in range(kpc):
        masks[j] = np.where(q >= j * 128 + p, 0.0, NEG).astype(np.float32)
    return masks


def _perm_head_rows(w):
    """Permute rows within each 128-row head block: evens first, odds second."""
    nh = w.shape[0] // HEAD_DIM
    return np.ascontiguousarray(
        w.reshape(nh, HEAD_DIM, -1)[:, _ROPE_PERM, :].reshape(w.shape)
    )


def _bf16(a):
    return np.ascontiguousarray(a.astype(ml_dtypes.bfloat16))


def _pmajor(a):
    """[T*128, F...] -> [128, T, F...] partition-major layout."""
    t = a.shape[0] // 128
    return np.ascontiguousarray(
        a.reshape(t, 128, *a.shape[1:]).swapaxes(0, 1)
    )


def make_core_inputs(x, freq_cis, wq, wk, wv, wo):
    """Build the 8 per-core input maps."""
    x = np.asarray(x, np.float32)
    wq = np.asarray(wq, np.float32)
    wk = np.asarray(wk, np.float32)
    wv = np.asarray(wv, np.float32)
    wo = np.asarray(wo, np.float32)
    t1, t2 = _prep_tables(freq_cis)
    masks = _prep_masks()
    ident = _bf16(np.eye(128, dtype=np.float32))
    IQ = HQ * HEAD_DIM

    in_maps = []
    for core in range(8):
        b, g = divmod(core, N_GROUPS)
        wq_g = _perm_head_rows(wq[g * IQ:(g + 1) * IQ])
        wk_g = _perm_head_rows(wk[g * HEAD_DIM:(g + 1) * HEAD_DIM])
        wv_g = wv[g * HEAD_DIM:(g + 1) * HEAD_DIM]
        wqT = _pmajor(_bf16(wq_g.T))
        wkvT = _pmajor(_bf16(np.concatenate([wk_g.T, wv_g.T], axis=1)))
        woT = _pmajor(_bf16(wo[:, g * IQ:(g + 1) * IQ].T))
        in_maps.append({
            "xT": _pmajor(_bf16(x[b].T)),
            "wqT": wqT,
            "wkvT": wkvT,
            "woT": woT,
            "t1": _pmajor(t1),
            "t2": _pmajor(t2),
            "masks": np.ascontiguousarray(masks.swapaxes(0, 1)),
            "ident": ident,
            "ones_col": _bf16(np.ones((128, 1), np.float32)),
            "ones_row": np.ones((1, 128), np.float32),
        })
    return in_maps


_CACHED_NC = None


def _get_nc():
    global _CACHED_NC
    if _CACHED_NC is None:
        from concourse import bacc

        nc = bacc.Bacc("TRN2", target_bir_lowering=False, debug=False)
        build_attention_core(nc)
        nc.compile()
        _CACHED_NC = nc
    return _CACHED_NC


def kernel(x, freq_cis, wq, wk, wv, wo):
    from concourse.bass_utils import run_bass_kernel_spmd

    nc = _get_nc()
    in_maps = make_core_inputs(x, freq_cis, wq, wk, wv, wo)
    res = run_bass_kernel_spmd(nc, in_maps, list(range(8)))
    out = np.zeros((B, S, DIM), dtype=np.float32)
    for core in range(8):
        b = core // N_GROUPS
        out[b] += res.results[core]["out_partial"]
    return out



# revision 9
# speedup vs baseline: 1.0011x; 1.0011x over previous
"""Trainium2 Bass kernel for GQA attention forward (B=2, S=2048, D=2048,
16 q-heads / 4 kv-heads, head_dim=128, RoPE, causal).

Sharding: 8 cores = 2 (batch) x 4 (kv-head groups). Each core computes its
batch's attention for one kv-head group (4 q-heads + 1 kv head) and a
row-parallel partial of the output projection; the host sums the 4 partials
per batch.

v2 structure:
- Phase A uses weight-stationary projections (stationary = weight d-tile,
  moving = x s-chunk) so Q/K come out of the PE directly in [e, s] layout --
  no per-head PE transposes. RoPE runs on bf16 SBUF tiles at 2x DVE rate.
- Phase B (attention) and phase C (output projection) are merged into one
  software pipeline: chunk c's out-projection matmuls interleave with chunk
  c+1's score/exp/AV stream so the PE fills the exp-bound bubbles.
- Output partials are written bf16 (host accumulates in fp32).
"""

import sys

if "/opt/trn_rl_repo" not in sys.path:
    sys.path.insert(0, "/opt/trn_rl_repo")

import numpy as np
import ml_dtypes

import concourse.bass as bass
import concourse.tile as tile
from concourse import mybir

F32 = mybir.dt.float32
F32R = mybir.dt.float32r
BF16 = mybir.dt.bfloat16

# Full-problem constants (per reference).
B, S, DIM = 2, 2048, 2048
N_HEADS, N_KV_HEADS, HEAD_DIM = 16, 4, 128
N_GROUPS = N_KV_HEADS          # tensor-parallel groups
HQ = N_HEADS // N_KV_HEADS     # q heads per group
NEG = -1e30


def build_attention_core(nc, S=S, D=DIM, HQ=HQ, HD=HEAD_DIM, CHUNK=512):
    """Emit the per-core attention program into `nc` (Tile framework).

    Inputs (ExternalInput dram tensors):
      xT     [128, D/128, S] bf16  activations, d-major (partition = d % 128)
      wqT    [128, D/128, HQ*HD] bf16  q projection, transposed, RoPE-permuted
      wkvT   [128, D/128, 2*HD] bf16   [wk^T | wv^T] (wk RoPE-permuted)
      woT    [128, HQ*HD/128, D] bf16  output projection slice, transposed
      t1T    [128, S] bf16  RoPE table, [e, s] layout (permuted-half)
      t2T    [128, S] f32   RoPE table (applied to half-swapped projection)
      masks  [128, CHUNK//128, CHUNK] f32 additive causal masks
      ident  [128, 128] bf16  identity for PE transposes
      pswap  [128, 128] bf16  half-swap permutation for RoPE
      ones_col [128,1] bf16 / ones_row [1,128] f32r
    Output:
      out_partial [S, D] bf16
    """
    n_st = S // 128        # s tiles
    n_dt = D // 128        # d tiles
    n_ch = S // CHUNK      # q chunks
    kpc = CHUNK // 128     # k-tiles per chunk
    n_dc = D // CHUNK      # d chunks (out-proj)
    spc = CHUNK // 128     # s-tiles per chunk
    IQ = HQ * HD

    x_d = nc.dram_tensor("xT", [128, n_dt, S], BF16, kind="ExternalInput")
    wqT_d = nc.dram_tensor("wqT", [128, n_dt, IQ], BF16, kind="ExternalInput")
    wkvT_d = nc.dram_tensor("wkvT", [128, n_dt, 2 * HD], BF16, kind="ExternalInput")
    woT_d = nc.dram_tensor("woT", [128, IQ // 128, D], BF16, kind="ExternalInput")
    t1T_d = nc.dram_tensor("t1T", [128, S], BF16, kind="ExternalInput")
    t2T_d = nc.dram_tensor("t2T", [128, S], F32, kind="ExternalInput")
    masks_d = nc.dram_tensor("masks", [128, kpc, CHUNK], F32, kind="ExternalInput")
    ident_d = nc.dram_tensor("ident", [128, 128], BF16, kind="ExternalInput")
    pswap_d = nc.dram_tensor("pswap", [128, 128], BF16, kind="ExternalInput")
    onesc_d = nc.dram_tensor("ones_col", [128, 1], BF16, kind="ExternalInput")
    onesr_d = nc.dram_tensor("ones_row", [1, 128], F32R, kind="ExternalInput")
    out_d = nc.dram_tensor("out_partial", [S, D], BF16, kind="ExternalOutput")

    scale = float(HD) ** -0.5

    with tile.TileContext(nc) as tc:
        with (
            tc.tile_pool(name="persist", bufs=1) as persist,
            tc.tile_pool(name="constB", bufs=1) as constB,
        ):
            qT_sb = persist.tile([128, HQ, S], BF16)    # [e, h, s]
            kT_sb = persist.tile([128, S], BF16)        # [e, s]
            v_sb = persist.tile([128, n_st, HD], BF16)  # [s_in_tile, s_tile, e]
            oT_sb = persist.tile([128, HQ, S], BF16)    # [e, h, s]

            # weights / tables (scalar queue; x + phase-B consts on sync queue)
            wq_sb = persist.tile([128, n_dt, IQ], BF16)
            nc.scalar.dma_start(out=wq_sb, in_=wqT_d[:])
            wkv_sb = persist.tile([128, n_dt, 2 * HD], BF16)
            nc.scalar.dma_start(out=wkv_sb, in_=wkvT_d[:])
            t1T_sb = persist.tile([128, S], BF16)
            nc.scalar.dma_start(out=t1T_sb, in_=t1T_d[:])
            t2T_sb = persist.tile([128, S], F32)
            nc.scalar.dma_start(out=t2T_sb, in_=t2T_d[:])
            ident = persist.tile([128, 128], BF16)
            nc.scalar.dma_start(out=ident, in_=ident_d[:])
            pswap_sb = persist.tile([128, 128], BF16)
            nc.scalar.dma_start(out=pswap_sb, in_=pswap_d[:])
            woT_sb = persist.tile([128, IQ // 128, D], BF16)
            nc.scalar.dma_start(out=woT_sb, in_=woT_d[:])

            masks_sb = constB.tile([128, kpc, CHUNK], F32)
            nc.sync.dma_start(out=masks_sb, in_=masks_d[:])
            ones_col = constB.tile([128, 1], BF16)
            nc.sync.dma_start(out=ones_col, in_=onesc_d[:])
            ones_row = constB.tile([1, 128], F32R)
            nc.sync.dma_start(out=ones_row, in_=onesr_d[:])

            # ---------------- Phase A: projections + RoPE -------------------
            with (
                tc.tile_pool(name="xslab", bufs=2) as xpool,
                tc.tile_pool(name="rope", bufs=2) as rope_pool,
                tc.tile_pool(name="psA", bufs=2, space="PSUM") as psA_pool,
                tc.tile_pool(name="psw", bufs=2, space="PSUM") as psw_pool,
                tc.tile_pool(name="pst", bufs=2, space="PSUM") as pst_pool,
            ):
                for sc in range(n_ch):
                    sl = slice(sc * CHUNK, (sc + 1) * CHUNK)
                    xs = xpool.tile([128, n_dt, CHUNK], BF16, tag="xs")
                    nc.sync.dma_start(out=xs, in_=x_d[:, :, sl])
                    # cols 0..3 = q heads, 4 = k, 5 = v
                    for col in range(6):
                        ps = psA_pool.tile([128, CHUNK], F32, tag="ps")
                        for dt in range(n_dt):
                            if col < 4:
                                w_ap = wq_sb[:, dt, col * 128:(col + 1) * 128]
                            elif col == 4:
                                w_ap = wkv_sb[:, dt, 0:HD]
                            else:
                                w_ap = wkv_sb[:, dt, HD:2 * HD]
                            nc.tensor.matmul(
                                ps, w_ap, xs[:, dt, :],
                                start=(dt == 0), stop=(dt == n_dt - 1),
                            )
                        if col < 5:
                            # RoPE in [e, s]: rot = p*t1T + swap_halves(p)*t2T.
                            # DVE lanes can't cross partitions, so the half
                            # swap runs as a PE permutation matmul.
                            raw = rope_pool.tile([128, CHUNK], BF16, tag="raw")
                            nc.scalar.copy(raw, ps)
                            ps_sw = psw_pool.tile([128, CHUNK], F32, tag="psw")
                            nc.tensor.matmul(
                                ps_sw, pswap_sb, raw, start=True, stop=True,
                            )
                            m1 = rope_pool.tile([128, CHUNK], BF16, tag="m1")
                            nc.vector.tensor_mul(m1, raw, t1T_sb[:, sl])
                            m2 = rope_pool.tile([128, CHUNK], BF16, tag="m2")
                            nc.vector.tensor_mul(m2, ps_sw, t2T_sb[:, sl])
                            dest = (qT_sb[:, col, sl] if col < 4
                                    else kT_sb[:, sl])
                            nc.vector.tensor_add(dest, m1, m2)
                        else:
                            # v: copy [e, s] then PE-transpose to [s, e]
                            vT = rope_pool.tile([128, CHUNK], BF16, tag="vT")
                            nc.scalar.copy(vT, ps)
                            for j in range(spc):
                                ps_t = pst_pool.tile([128, 128], BF16, tag="pst")
                                nc.tensor.transpose(
                                    ps_t, vT[:, j * 128:(j + 1) * 128], ident
                                )
                                nc.vector.tensor_copy(
                                    v_sb[:, sc * spc + j, :], ps_t
                                )

            # ---------------- Phases B+C merged -----------------------------
            with (
                tc.tile_pool(name="expt", bufs=4) as expt_pool,
                tc.tile_pool(name="maskbuf", bufs=3) as mask_pool,
                tc.tile_pool(name="sums", bufs=2) as sums_pool,
                tc.tile_pool(name="recip", bufs=2) as rec_pool,
                tc.tile_pool(name="outsb", bufs=2) as outsb_pool,
                tc.tile_pool(name="ps_s", bufs=2, space="PSUM") as pss_pool,
                tc.tile_pool(name="ps_o", bufs=2, space="PSUM") as pso_pool,
                tc.tile_pool(name="ps_sum", bufs=2, space="PSUM") as pssum_pool,
                tc.tile_pool(name="ps_x", bufs=2, space="PSUM") as psx_pool,
            ):
                norm_pending = [None]
                cq = []           # pending out-proj groups: (c, st_in_chunk)
                out_tiles = {}    # st_in_chunk -> out_sb tile (current chunk)

                def emit_norm():
                    ps_o_, ps_sum_, h_, c_ = norm_pending[0]
                    norm_pending[0] = None
                    sums_sb = sums_pool.tile([1, CHUNK], F32R, tag="sums")
                    with nc.allow_low_precision(reason="f32r denom"):
                        nc.vector.tensor_copy(sums_sb, ps_sum_)
                    ps_b = psx_pool.tile([128, CHUNK], F32, tag="psx")
                    nc.tensor.matmul(
                        ps_b, ones_row, sums_sb, start=True, stop=True,
                    )
                    recip = rec_pool.tile([128, CHUNK], F32, tag="recip")
                    nc.vector.reciprocal_approx_fast(recip, ps_b)
                    nc.vector.tensor_mul(
                        oT_sb[:, h_, c_ * CHUNK:(c_ + 1) * CHUNK],
                        ps_o_, recip,
                    )
                    if h_ == HQ - 1:
                        # all heads of chunk c_ normalized: queue out-proj
                        for st in range(spc):
                            cq.append((c_, st))

                def emit_c_group():
                    c_, sti = cq.pop(0)
                    st = c_ * spc + sti
                    st_sl = slice(st * 128, (st + 1) * 128)
                    out_sb = outsb_pool.tile([128, D], BF16, tag="outsb")
                    for dc in range(n_dc):
                        ps_d = psx_pool.tile([128, CHUNK], F32, tag="psx")
                        for it in range(HQ):
                            nc.tensor.matmul(
                                ps_d,
                                oT_sb[:, it, st_sl],
                                woT_sb[:, it, dc * CHUNK:(dc + 1) * CHUNK],
                                start=(it == 0), stop=(it == HQ - 1),
                            )
                        nc.scalar.copy(
                            out_sb[:, dc * CHUNK:(dc + 1) * CHUNK], ps_d
                        )
                    nc.sync.dma_start(out=out_d[st_sl, :], in_=out_sb)

                for c in range(n_ch):
                    c_sl = slice(c * CHUNK, (c + 1) * CHUNK)
                    for h in range(HQ):
                        ps_o = pso_pool.tile([128, CHUNK], F32, tag="ps_o")
                        ps_sum = pssum_pool.tile([1, CHUNK], F32, tag="ps_sum")
                        n_kj = (c + 1) * kpc
                        pending = []

                        def flush_one():
                            pe, pj, poff = pending.pop(0)
                            nc.tensor.matmul(
                                ps_o[:, poff:], v_sb[:, pj, :], pe,
                                start=(pj == 0), stop=(pj == n_kj - 1),
                            )
                            nc.tensor.matmul(
                                ps_sum[:, poff:], ones_col, pe,
                                start=(pj == 0), stop=(pj == n_kj - 1),
                            )

                        for kj in range(n_kj):
                            # columns left of the diagonal block are fully
                            # masked: skip them (q >= kj*128 only)
                            off = max(0, (kj - c * kpc)) * 128
                            w = CHUNK - off
                            ps_s = pss_pool.tile([128, CHUNK], F32, tag="ps_s")
                            nc.tensor.matmul(
                                ps_s[:, 0:w],
                                kT_sb[:, kj * 128:(kj + 1) * 128],
                                qT_sb[:, h, c * CHUNK + off:(c + 1) * CHUNK],
                                start=True, stop=True,
                            )
                            if kj >= c * kpc:  # diagonal chunk: causal mask
                                msk = mask_pool.tile([128, CHUNK], F32, tag="msk")
                                nc.vector.tensor_add(
                                    msk[:, 0:w], ps_s[:, 0:w],
                                    masks_sb[:, kj % kpc, off:],
                                )
                                exp_in = msk
                            else:
                                exp_in = ps_s
                            expT = expt_pool.tile([128, CHUNK], BF16, tag="expT")
                            nc.scalar.activation(
                                expT[:, 0:w], exp_in[:, 0:w],
                                mybir.ActivationFunctionType.Exp,
                                scale=scale,
                            )
                            pending.append((expT[:, 0:w], kj, off))
                            if kj == 1 and norm_pending[0] is not None:
                                emit_norm()
                            if kj > 1 and cq:
                                emit_c_group()
                            if len(pending) > 2:
                                flush_one()
                        while pending:
                            flush_one()
                        norm_pending[0] = (ps_o, ps_sum, h, c)

                emit_norm()
                while cq:
                    emit_c_group()

    return nc


# ---------------------------------------------------------------------------
# Host-side prep


_ROPE_PERM = np.concatenate([np.arange(0, HEAD_DIM, 2), np.arange(1, HEAD_DIM, 2)])


def _prep_tables(freq_cis, S_=S, HD_=HEAD_DIM):
    """RoPE tables in permuted-half layout: rot = q*t1 + swap(q)*t2."""
    fc = np.asarray(freq_cis, dtype=np.float32)
    A = fc[:, :, 0, 0]
    Bm = fc[:, :, 0, 1]
    C = fc[:, :, 1, 0]
    Dm = fc[:, :, 1, 1]
    t1 = np.concatenate([A, Dm], axis=1).astype(np.float32)  # [S, HD]
    t2 = np.concatenate([Bm, C], axis=1).astype(np.float32)
    return np.ascontiguousarray(t1), np.ascontiguousarray(t2)


def _prep_masks(chunk=512):
    kpc = chunk // 128
    masks = np.zeros((kpc, 128, chunk), dtype=np.float32)
    q = np.arange(chunk)[None, :]
    p = np.arange(128)[:, None]
    for j in range(kpc):
        masks[j] = np.where(q >= j * 128 + p, 0.0, NEG).astype(np.float32)
    return masks


def _perm_head_rows(w):
    """Permute rows within each 128-row head block: evens first, odds second."""
    nh = w.shape[0] // HEAD_DIM
    return np.ascontiguousarray(
        w.reshape(nh, HEAD_DIM, -1)[:, _ROPE_PERM, :].reshape(w.shape)
    )


def _bf16(a):
    return np.ascontiguousarray(a.astype(ml_dtypes.bfloat16))


def _pmajor(a):
    """[T*128, F...] -> [128, T, F...] partition-major layout."""
    t = a.shape[0] // 128
    return np.ascontiguousarray(
        a.reshape(t, 128, *a.shape[1:]).swapaxes(0, 1)
    )


def make_core_inputs(x, freq_cis, wq, wk, wv, wo):
    """Build the 8 per-core input maps."""
    x = np.asarray(x, np.float32)
    wq = np.asarray(wq, np.float32)
    wk = np.asarray(wk, np.float32)
    wv = np.asarray(wv, np.float32)
    wo = np.asarray(wo, np.float32)
    t1, t2 = _prep_tables(freq_cis)
    t1T = _bf16(t1.T)   # [HD, S] = [e, s]
    t2T = np.ascontiguousarray(t2.T)
    masks = _prep_masks()
    ident = _bf16(np.eye(128, dtype=np.float32))
    pswap = _bf16(np.roll(np.eye(128, dtype=np.float32), 64, axis=1))
    IQ = HQ * HEAD_DIM

    in_maps = []
    for core in range(8):
        b, g = divmod(core, N_GROUPS)
        wq_g = _perm_head_rows(wq[g * IQ:(g + 1) * IQ])
        wk_g = _perm_head_rows(wk[g * HEAD_DIM:(g + 1) * HEAD_DIM])
        wv_g = wv[g * HEAD_DIM:(g + 1) * HEAD_DIM]
        wqT = _pmajor(_bf16(wq_g.T))
        wkvT = _pmajor(_bf16(np.concatenate([wk_g.T, wv_g.T], axis=1)))
        woT = _pmajor(_bf16(wo[:, g * IQ:(g + 1) * IQ].T))
        in_maps.append({
            "xT": _pmajor(_bf16(x[b].T)),
            "wqT": wqT,
            "wkvT": wkvT,
            "woT": woT,
            "t1T": t1T,
            "t2T": t2T,
            "masks": np.ascontiguousarray(masks.swapaxes(0, 1)),
            "ident": ident,
            "pswap": pswap,
            "ones_col": _bf16(np.ones((128, 1), np.float32)),
            "ones_row": np.ones((1, 128), np.float32),
        })
    return in_maps


_CACHED_NC = None


def _get_nc():
    global _CACHED_NC
    if _CACHED_NC is None:
        from concourse import bacc

        nc = bacc.Bacc("TRN2", target_bir_lowering=False, debug=False)
        build_attention_core(nc)
        nc.compile()
        _CACHED_NC = nc
    return _CACHED_NC


def kernel(x, freq_cis, wq, wk, wv, wo):
    from concourse.bass_utils import run_bass_kernel_spmd

    nc = _get_nc()
    in_maps = make_core_inputs(x, freq_cis, wq, wk, wv, wo)
    res = run_bass_kernel_spmd(nc, in_maps, list(range(8)))
    out = np.zeros((B, S, DIM), dtype=np.float32)
    for core in range(8):
        b = core // N_GROUPS
        out[b] += res.results[core]["out_partial"].astype(np.float32)
    return out


# revision 21
# speedup vs baseline: 1.0250x; 1.0239x over previous
"""Trainium2 Bass kernel for GQA attention forward (B=2, S=2048, D=2048,
16 q-heads / 4 kv-heads, head_dim=128, RoPE, causal).

Sharding: 8 cores = 2 (batch) x 4 (kv-head groups). Each core computes its
batch's attention for one kv-head group (4 q-heads + 1 kv head) and a
row-parallel partial of the output projection; the host sums the 4 partials
per batch.

v2 structure:
- Phase A uses weight-stationary projections (stationary = weight d-tile,
  moving = x s-chunk) so Q/K come out of the PE directly in [e, s] layout --
  no per-head PE transposes. RoPE runs on bf16 SBUF tiles at 2x DVE rate.
- Phase B (attention) and phase C (output projection) are merged into one
  software pipeline: chunk c's out-projection matmuls interleave with chunk
  c+1's score/exp/AV stream so the PE fills the exp-bound bubbles.
- Output partials are written bf16 (host accumulates in fp32).
"""

import sys

if "/opt/trn_rl_repo" not in sys.path:
    sys.path.insert(0, "/opt/trn_rl_repo")

import numpy as np
import ml_dtypes

import concourse.bass as bass
import concourse.tile as tile
from concourse import mybir

F32 = mybir.dt.float32
F32R = mybir.dt.float32r
BF16 = mybir.dt.bfloat16

# Full-problem constants (per reference).
B, S, DIM = 2, 2048, 2048
N_HEADS, N_KV_HEADS, HEAD_DIM = 16, 4, 128
N_GROUPS = N_KV_HEADS          # tensor-parallel groups
HQ = N_HEADS // N_KV_HEADS     # q heads per group
NEG = -1e30


def build_attention_core(nc, S=S, D=DIM, HQ=HQ, HD=HEAD_DIM, CHUNK=512):
    """Emit the per-core attention program into `nc` (Tile framework).

    Inputs (ExternalInput dram tensors):
      xT     [128, D/128, S] bf16  activations, d-major (partition = d % 128)
      wqT    [128, D/128, HQ*HD] bf16  q projection, transposed, RoPE-permuted
      wkvT   [128, D/128, 2*HD] bf16   [wk^T | wv^T] (wk RoPE-permuted)
      woT    [128, HQ*HD/128, D] bf16  output projection slice, transposed
      t1T    [128, S] bf16  RoPE table, [e, s] layout (permuted-half)
      t2T    [128, S] f32   RoPE table (applied to half-swapped projection)
      masks  [128, CHUNK//128, CHUNK] f32 additive causal masks
      ident  [128, 128] bf16  identity for PE transposes
      pswap  [128, 128] bf16  half-swap permutation for RoPE
      ones_col [128,1] bf16 (col-tiled denominator matmuls)
      sel    [128, 128] f32r  rows {0,32,64,96} = 1: reduces the 4 col-tiled
             denominator partials and broadcasts across partitions
    Output:
      out_partial [S, D] bf16
    """
    n_st = S // 128        # s tiles
    n_dt = D // 128        # d tiles
    n_ch = S // CHUNK      # q chunks
    kpc = CHUNK // 128     # k-tiles per chunk
    n_dc = D // CHUNK      # d chunks (out-proj)
    spc = CHUNK // 128     # s-tiles per chunk
    IQ = HQ * HD

    x_d = nc.dram_tensor("xT", [128, n_dt, S], BF16, kind="ExternalInput")
    wqT_d = nc.dram_tensor("wqT", [128, n_dt, IQ], BF16, kind="ExternalInput")
    wkvT_d = nc.dram_tensor("wkvT", [128, n_dt, 2 * HD], BF16, kind="ExternalInput")
    woT_d = nc.dram_tensor("woT", [128, IQ // 128, D], BF16, kind="ExternalInput")
    t1T_d = nc.dram_tensor("t1T", [128, S], BF16, kind="ExternalInput")
    t2T_d = nc.dram_tensor("t2T", [128, S], F32, kind="ExternalInput")
    masks_d = nc.dram_tensor("masks", [128, kpc, CHUNK], F32, kind="ExternalInput")
    ident_d = nc.dram_tensor("ident", [128, 128], BF16, kind="ExternalInput")
    pswap_d = nc.dram_tensor("pswap", [128, 128], BF16, kind="ExternalInput")
    onesc_d = nc.dram_tensor("ones_col", [128, 1], BF16, kind="ExternalInput")
    sel_d = nc.dram_tensor("sel", [128, 128], F32R, kind="ExternalInput")
    out_d = nc.dram_tensor("out_partial", [S, D], BF16, kind="ExternalOutput")

    scale = float(HD) ** -0.5

    with tile.TileContext(nc) as tc:
        with (
            tc.tile_pool(name="persist", bufs=1) as persist,
            tc.tile_pool(name="constB", bufs=1) as constB,
        ):
            qT_sb = persist.tile([128, HQ, S], BF16)    # [e, h, s]
            kT_sb = persist.tile([128, S], BF16)        # [e, s]
            v_sb = persist.tile([128, n_st, HD], BF16)  # [s_in_tile, s_tile, e]
            oT_sb = persist.tile([128, HQ, S], BF16)    # [e, h, s]

            # weights / tables on the scalar queue, split so the first
            # matmuls only wait on their own slice; x on the sync queue.
            wq_sb = persist.tile([128, n_dt, IQ], BF16)
            for hh in range(HQ):
                nc.scalar.dma_start(
                    out=wq_sb[:, :, hh * HD:(hh + 1) * HD],
                    in_=wqT_d[:, :, hh * HD:(hh + 1) * HD],
                )
            wkv_sb = persist.tile([128, n_dt, 2 * HD], BF16)
            nc.scalar.dma_start(
                out=wkv_sb[:, :, 0:HD], in_=wkvT_d[:, :, 0:HD]
            )
            nc.scalar.dma_start(
                out=wkv_sb[:, :, HD:2 * HD], in_=wkvT_d[:, :, HD:2 * HD]
            )
            t1T_sb = persist.tile([128, S], BF16)
            nc.scalar.dma_start(out=t1T_sb, in_=t1T_d[:])
            t2T_sb = persist.tile([128, S], F32)
            nc.scalar.dma_start(out=t2T_sb, in_=t2T_d[:])
            ident = persist.tile([128, 128], BF16)
            nc.scalar.dma_start(out=ident, in_=ident_d[:])
            pswap_sb = persist.tile([128, 128], BF16)
            nc.scalar.dma_start(out=pswap_sb, in_=pswap_d[:])

            masks_sb = constB.tile([128, kpc, CHUNK], F32)
            nc.scalar.dma_start(out=masks_sb, in_=masks_d[:])
            ones_col = constB.tile([128, 1], BF16)
            nc.scalar.dma_start(out=ones_col, in_=onesc_d[:])
            sel_sb = constB.tile([128, 128], F32R)
            nc.scalar.dma_start(out=sel_sb, in_=sel_d[:])
            woT_sb = persist.tile([128, IQ // 128, D], BF16)
            nc.scalar.dma_start(out=woT_sb, in_=woT_d[:])

            # ---------------- Phase A: projections + RoPE -------------------
            with (
                tc.tile_pool(name="xslab", bufs=2) as xpool,
                tc.tile_pool(name="rope", bufs=2) as rope_pool,
                tc.tile_pool(name="psA", bufs=2, space="PSUM") as psA_pool,
                tc.tile_pool(name="psw", bufs=2, space="PSUM") as psw_pool,
                tc.tile_pool(name="pst", bufs=2, space="PSUM") as pst_pool,
            ):
                for sc in range(n_ch):
                    sl = slice(sc * CHUNK, (sc + 1) * CHUNK)
                    xs = xpool.tile([128, n_dt, CHUNK], BF16, tag="xs")
                    nc.sync.dma_start(out=xs, in_=x_d[:, :, sl])
                    # cols 0..3 = q heads, 4 = k, 5 = v
                    for col in range(6):
                        ps = psA_pool.tile([128, CHUNK], F32, tag="ps")
                        for dt in range(n_dt):
                            if col < 4:
                                w_ap = wq_sb[:, dt, col * 128:(col + 1) * 128]
                            elif col == 4:
                                w_ap = wkv_sb[:, dt, 0:HD]
                            else:
                                w_ap = wkv_sb[:, dt, HD:2 * HD]
                            nc.tensor.matmul(
                                ps, w_ap, xs[:, dt, :],
                                start=(dt == 0), stop=(dt == n_dt - 1),
                            )
                        if col < 5:
                            # RoPE in [e, s]: rot = p*t1T + swap_halves(p)*t2T.
                            # DVE lanes can't cross partitions, so the half
                            # swap runs as a PE permutation matmul.
                            raw = rope_pool.tile([128, CHUNK], BF16, tag="raw")
                            nc.scalar.copy(raw, ps)
                            ps_sw = psw_pool.tile([128, CHUNK], F32, tag="psw")
                            nc.tensor.matmul(
                                ps_sw, pswap_sb, raw, start=True, stop=True,
                            )
                            m1 = rope_pool.tile([128, CHUNK], BF16, tag="m1")
                            nc.vector.tensor_mul(m1, raw, t1T_sb[:, sl])
                            m2 = rope_pool.tile([128, CHUNK], BF16, tag="m2")
                            nc.vector.tensor_mul(m2, ps_sw, t2T_sb[:, sl])
                            dest = (qT_sb[:, col, sl] if col < 4
                                    else kT_sb[:, sl])
                            nc.vector.tensor_add(dest, m1, m2)
                        else:
                            # v: copy [e, s] then PE-transpose to [s, e]
                            vT = rope_pool.tile([128, CHUNK], BF16, tag="vT")
                            nc.scalar.copy(vT, ps)
                            for j in range(spc):
                                ps_t = pst_pool.tile([128, 128], BF16, tag="pst")
                                nc.tensor.transpose(
                                    ps_t, vT[:, j * 128:(j + 1) * 128], ident
                                )
                                nc.vector.tensor_copy(
                                    v_sb[:, sc * spc + j, :], ps_t
                                )


            # ---------------- Phases B+C merged -----------------------------
            with (
                tc.tile_pool(name="expt", bufs=4) as expt_pool,
                tc.tile_pool(name="maskbuf", bufs=3) as mask_pool,
                tc.tile_pool(name="sums", bufs=2) as sums_pool,
                tc.tile_pool(name="recip", bufs=2) as rec_pool,
                tc.tile_pool(name="outsb", bufs=2) as outsb_pool,
                tc.tile_pool(name="ps_s", bufs=2, space="PSUM") as pss_pool,
                tc.tile_pool(name="ps_o", bufs=2, space="PSUM") as pso_pool,
                tc.tile_pool(name="ps_sum", bufs=2, space="PSUM") as pssum_pool,
                tc.tile_pool(name="ps_x", bufs=2, space="PSUM") as psx_pool,
            ):
                norm_pending = [None]
                cq = []           # pending out-proj groups: (c, st_in_chunk)
                pssum_seeded = [0]

                def emit_norm():
                    ps_o_, ps_sum_, h_, c_ = norm_pending[0]
                    norm_pending[0] = None
                    sums_sb = sums_pool.tile([128, CHUNK], F32R, tag="sums")
                    with nc.allow_low_precision(reason="f32r denom"):
                        nc.vector.tensor_copy(sums_sb, ps_sum_)
                    # sel has ones at rows {0,32,64,96}: one matmul both
                    # reduces the 4 col-tiled partials and broadcasts the
                    # denominator across partitions.
                    ps_b = psx_pool.tile([128, CHUNK], F32, tag="psx")
                    nc.tensor.matmul(
                        ps_b, sel_sb, sums_sb, start=True, stop=True,
                    )
                    recip = rec_pool.tile([128, CHUNK], F32, tag="recip")
                    nc.vector.reciprocal_approx_fast(recip, ps_b)
                    nc.vector.tensor_mul(
                        oT_sb[:, h_, c_ * CHUNK:(c_ + 1) * CHUNK],
                        ps_o_, recip,
                    )
                    if h_ == HQ - 1:
                        # all heads of chunk c_ normalized: queue out-proj
                        for st in range(spc):
                            cq.append((c_, st))

                def emit_c_group():
                    c_, sti = cq.pop(0)
                    st = c_ * spc + sti
                    st_sl = slice(st * 128, (st + 1) * 128)
                    out_sb = outsb_pool.tile([128, D], BF16, tag="outsb")
                    for dc in range(n_dc):
                        ps_d = psx_pool.tile([128, CHUNK], F32, tag="psx")
                        for it in range(HQ):
                            nc.tensor.matmul(
                                ps_d,
                                oT_sb[:, it, st_sl],
                                woT_sb[:, it, dc * CHUNK:(dc + 1) * CHUNK],
                                start=(it == 0), stop=(it == HQ - 1),
                            )
                        nc.vector.tensor_copy(
                            out_sb[:, dc * CHUNK:(dc + 1) * CHUNK], ps_d
                        )
                    nc.sync.dma_start(out=out_d[st_sl, :], in_=out_sb)

                for c in range(n_ch):
                    c_sl = slice(c * CHUNK, (c + 1) * CHUNK)
                    for h in range(HQ):
                        ps_o = pso_pool.tile([128, CHUNK], F32, tag="ps_o")
                        ps_sum = pssum_pool.tile([128, CHUNK], F32, tag="ps_sum")
                        if pssum_seeded[0] < 2:
                            # first use of each ring bank: zero the rows the
                            # col-tiled sums matmuls don't write, so the sel
                            # reduce never multiplies 0 by PSUM garbage
                            nc.vector.memset(ps_sum, 0.0)
                            pssum_seeded[0] += 1
                        n_kj = (c + 1) * kpc
                        pending = []
                        sgroup = []

                        def flush_av():
                            pe, pj, poff = pending.pop(0)
                            nc.tensor.matmul(
                                ps_o[:, poff:], v_sb[:, pj, :], pe,
                                start=(pj == 0), stop=(pj == n_kj - 1),
                            )

                        def flush_sums():
                            # 4 concurrent M=1 matmuls in distinct column
                            # groups of the PE array
                            for i, (pe, pj, poff) in enumerate(sgroup):
                                nc.tensor.matmul(
                                    ps_sum[32 * i:32 * i + 1, poff:],
                                    ones_col, pe,
                                    start=(pj < 4), stop=(pj >= n_kj - 4),
                                    tile_position=(0, 32 * i),
                                )
                            sgroup.clear()

                        for kj in range(n_kj):
                            # columns left of the diagonal block are fully
                            # masked: skip them (q >= kj*128 only)
                            off = max(0, (kj - c * kpc)) * 128
                            w = CHUNK - off
                            ps_s = pss_pool.tile([128, CHUNK], F32, tag="ps_s")
                            nc.tensor.matmul(
                                ps_s[:, 0:w],
                                kT_sb[:, kj * 128:(kj + 1) * 128],
                                qT_sb[:, h, c * CHUNK + off:(c + 1) * CHUNK],
                                start=True, stop=True,
                            )
                            if kj >= c * kpc:  # diagonal chunk: causal mask
                                msk = mask_pool.tile([128, CHUNK], F32, tag="msk")
                                nc.vector.tensor_add(
                                    msk[:, 0:w], ps_s[:, 0:w],
                                    masks_sb[:, kj % kpc, off:],
                                )
                                exp_in = msk
                            else:
                                exp_in = ps_s
                            expT = expt_pool.tile([128, CHUNK], BF16, tag="expT")
                            nc.scalar.activation(
                                expT[:, 0:w], exp_in[:, 0:w],
                                mybir.ActivationFunctionType.Exp,
                                scale=scale,
                            )
                            pending.append((expT[:, 0:w], kj, off))
                            sgroup.append((expT[:, 0:w], kj, off))
                            if len(sgroup) == 4:
                                flush_sums()
                            if kj == 1 and norm_pending[0] is not None:
                                emit_norm()
                            if kj > 1 and cq:
                                emit_c_group()
                            if len(pending) > 2:
                                flush_av()
                        while pending:
                            flush_av()
                        norm_pending[0] = (ps_o, ps_sum, h, c)

                emit_norm()
                while cq:
                    emit_c_group()

    return nc


# ---------------------------------------------------------------------------
# Host-side prep


_ROPE_PERM = np.concatenate([np.arange(0, HEAD_DIM, 2), np.arange(1, HEAD_DIM, 2)])


def _prep_tables(freq_cis, S_=S, HD_=HEAD_DIM):
    """RoPE tables in permuted-half layout: rot = q*t1 + swap(q)*t2."""
    fc = np.asarray(freq_cis, dtype=np.float32)
    A = fc[:, :, 0, 0]
    Bm = fc[:, :, 0, 1]
    C = fc[:, :, 1, 0]
    Dm = fc[:, :, 1, 1]
    t1 = np.concatenate([A, Dm], axis=1).astype(np.float32)  # [S, HD]
    t2 = np.concatenate([Bm, C], axis=1).astype(np.float32)
    return np.ascontiguousarray(t1), np.ascontiguousarray(t2)


def _prep_masks(chunk=512):
    kpc = chunk // 128
    masks = np.zeros((kpc, 128, chunk), dtype=np.float32)
    q = np.arange(chunk)[None, :]
    p = np.arange(128)[:, None]
    for j in range(kpc):
        masks[j] = np.where(q >= j * 128 + p, 0.0, NEG).astype(np.float32)
    return masks


def _perm_head_rows(w):
    """Permute rows within each 128-row head block: evens first, odds second."""
    nh = w.shape[0] // HEAD_DIM
    return np.ascontiguousarray(
        w.reshape(nh, HEAD_DIM, -1)[:, _ROPE_PERM, :].reshape(w.shape)
    )


def _bf16(a):
    return np.ascontiguousarray(a.astype(ml_dtypes.bfloat16))


def _pmajor(a):
    """[T*128, F...] -> [128, T, F...] partition-major layout."""
    t = a.shape[0] // 128
    return np.ascontiguousarray(
        a.reshape(t, 128, *a.shape[1:]).swapaxes(0, 1)
    )


def make_core_inputs(x, freq_cis, wq, wk, wv, wo):
    """Build the 8 per-core input maps."""
    x = np.asarray(x, np.float32)
    wq = np.asarray(wq, np.float32)
    wk = np.asarray(wk, np.float32)
    wv = np.asarray(wv, np.float32)
    wo = np.asarray(wo, np.float32)
    t1, t2 = _prep_tables(freq_cis)
    t1T = _bf16(t1.T)   # [HD, S] = [e, s]
    t2T = np.ascontiguousarray(t2.T)
    masks = _prep_masks()
    ident = _bf16(np.eye(128, dtype=np.float32))
    pswap = _bf16(np.roll(np.eye(128, dtype=np.float32), 64, axis=1))
    sel = np.zeros((128, 128), dtype=np.float32)
    sel[[0, 32, 64, 96], :] = 1.0
    IQ = HQ * HEAD_DIM

    in_maps = []
    for core in range(8):
        b, g = divmod(core, N_GROUPS)
        wq_g = _perm_head_rows(wq[g * IQ:(g + 1) * IQ])
        wk_g = _perm_head_rows(wk[g * HEAD_DIM:(g + 1) * HEAD_DIM])
        wv_g = wv[g * HEAD_DIM:(g + 1) * HEAD_DIM]
        wqT = _pmajor(_bf16(wq_g.T))
        wkvT = _pmajor(_bf16(np.concatenate([wk_g.T, wv_g.T], axis=1)))
        woT = _pmajor(_bf16(wo[:, g * IQ:(g + 1) * IQ].T))
        in_maps.append({
            "xT": _pmajor(_bf16(x[b].T)),
            "wqT": wqT,
            "wkvT": wkvT,
            "woT": woT,
            "t1T": t1T,
            "t2T": t2T,
            "masks": np.ascontiguousarray(masks.swapaxes(0, 1)),
            "ident": ident,
            "pswap": pswap,
            "ones_col": _bf16(np.ones((128, 1), np.float32)),
            "sel": sel,
        })
    return in_maps


_CACHED_NC = None


def _get_nc():
    global _CACHED_NC
    if _CACHED_NC is None:
        from concourse import bacc

        nc = bacc.Bacc("TRN2", target_bir_lowering=False, debug=False)
        build_attention_core(nc)
        nc.compile()
        _CACHED_NC = nc
    return _CACHED_NC


def kernel(x, freq_cis, wq, wk, wv, wo):
    from concourse.bass_utils import run_bass_kernel_spmd

    nc = _get_nc()
    in_maps = make_core_inputs(x, freq_cis, wq, wk, wv, wo)
    res = run_bass_kernel_spmd(nc, in_maps, list(range(8)))
    out = np.zeros((B, S, DIM), dtype=np.float32)
    for core in range(8):
        b = core // N_GROUPS
        out[b] += res.results[core]["out_partial"].astype(np.float32)
    return out
